# revision 8
# baseline (speedup 1.0000x reference)
"""Causal GQA attention block (RoPE, 16 q-heads / 4 kv-heads, D=1024, S=2048, B=2)
distributed over 8 NeuronCores: data-parallel over batch (2) x tensor-parallel
over kv-groups (4). Each core computes 4 query heads + 1 kv head for one batch
element, Megatron-style: Wq/Wk/Wv column-parallel, Wo row-parallel with the
row-parallel partial sums reduced on host.

v2 layout notes (per core):
 - bf16 everywhere on the PE (fp32 PSUM accumulate); host pre-converts
   weights/xT/tables to bf16; outputs written bf16, host upcasts + reduces.
 - Everything transposed: xT [D, S], QT per head pair [128, S] (2 heads
   stacked on partitions), KT zero-padded in both partition halves
   (kt_lo=[K;0], kt_hi=[0;K]) for row-packed score matmuls, scores^T [j, i]
   so AV contracts j on partitions.
 - Attention runs two passes per i-tile (kt_lo then kt_hi). Per piece the
   scores of both w-pairs land in one [128, 2, 512] PSUM tile -> ONE
   1024-wide Exp activation. PSUM: bg(2) + sc(2x2) + av(2) = 8 banks.
 - Softmax denominator fused into AV: lhsT = [V | ones] (M=128) so psum rows
   64:128 hold the replicated denominator; reciprocal+multiply then Wo.
 - RoPE rotate_half via PE matmul with a +-1 permutation matrix; mul/add on
   DVE in bf16.
 - V natural layout obtained with dma_start_transpose (no PE/psum involved).
"""

import os
import sys
import types

import numpy as np

import concourse.bass as bass
import concourse.mybir as mybir
import concourse.tile as tile
from concourse import bacc
from concourse.bass_utils import run_bass_kernel_spmd

F32 = mybir.dt.float32
BF16 = mybir.dt.bfloat16
AF = mybir.ActivationFunctionType

B, S, D = 2, 2048, 1024
H, KV, HD = 16, 4, 64
NH = 4  # query heads per core
P = 128
NT = S // 512  # 4 i-tiles of 512
KC = D // P  # 8 contraction chunks
JC = S // P  # 16 j-chunks
N_CORES = 8

_cached = {}


def _install_trace_hook():
    """NTFF profiling hook shim (the container's antenv lacks axon_hooks)."""
    try:
        import antenv

        if "antenv.axon_hooks" in sys.modules:
            return
        mod = types.ModuleType("antenv.axon_hooks")
        _h = [None]
        mod.set_axon_ntff_profile_hook = lambda h: _h.__setitem__(0, h)
        mod.get_axon_ntff_profile_hook = lambda: _h[0]
        sys.modules["antenv.axon_hooks"] = mod
        antenv.axon_hooks = mod
        from trn_agent_boot.trn_boot import _ntff_profile_via_ctypes

        mod.set_axon_ntff_profile_hook(
            _ntff_profile_via_ctypes("/opt/axon/libaxon_pjrt.so")
        )
    except Exception:
        pass


def build_bass():
    nc = bacc.Bacc("TRN2", target_bir_lowering=False, debug=False, num_devices=N_CORES)

    xT = nc.dram_tensor("xT", [D, S], BF16, kind="ExternalInput")
    wq = nc.dram_tensor("wq", [D, NH * HD], BF16, kind="ExternalInput")
    wkv = nc.dram_tensor("wkv", [D, 2 * HD], BF16, kind="ExternalInput")
    wo = nc.dram_tensor("wo", [NH * HD, D], BF16, kind="ExternalInput")
    cos2 = nc.dram_tensor("cos2", [P, S], BF16, kind="ExternalInput")
    sin2 = nc.dram_tensor("sin2", [P, S], BF16, kind="ExternalInput")
    r2t = nc.dram_tensor("r2t", [P, P], BF16, kind="ExternalInput")
    tri = nc.dram_tensor("tri", [P, P], BF16, kind="ExternalInput")
    out = nc.dram_tensor("out", [S, D], BF16, kind="ExternalOutput")

    with tile.TileContext(nc) as tc:
        with (
            tc.tile_pool(name="const", bufs=1) as const,
            tc.tile_pool(name="persist", bufs=1) as persist,
            tc.tile_pool(name="sb_tmp", bufs=2) as sb_tmp,
            tc.tile_pool(name="sb_pt", bufs=4) as sb_pt,
            tc.tile_pool(name="sb_ot", bufs=2) as sb_ot,
            tc.tile_pool(name="sb_out", bufs=2) as sb_out,
            tc.tile_pool(name="ps", bufs=1, space="PSUM") as ps,
        ):
            # ---- constants / weights (DMA order = need order) ----
            xT_sb = [
                persist.tile([P, S], BF16, tag=f"xT{k}", name=f"xT_sb{k}")
                for k in range(KC)
            ]
            wkv_sb = const.tile([P, KC, 2 * HD], BF16)
            nc.sync.dma_start(wkv_sb[:], wkv.rearrange("(k p) m -> p k m", p=P))
            nc.sync.dma_start(xT_sb[0][:], xT[0:P, :])
            wq_sb = const.tile([P, KC, NH * HD], BF16)
            nc.sync.dma_start(wq_sb[:], wq.rearrange("(k p) m -> p k m", p=P))
            cos_sb = const.tile([P, S], BF16)
            sin_sb = const.tile([P, S], BF16)
            nc.sync.dma_start(cos_sb[:], cos2[:])
            nc.sync.dma_start(sin_sb[:], sin2[:])
            r2t_sb = const.tile([P, P], BF16)
            nc.sync.dma_start(r2t_sb[:], r2t[:])
            for k in range(1, KC):
                nc.sync.dma_start(xT_sb[k][:], xT[k * P : (k + 1) * P, :])
            tri2_sb = const.tile([P, 2, P], BF16)
            nc.sync.dma_start(tri2_sb[:, 0, :], tri[:])
            nc.sync.dma_start(tri2_sb[:, 1, :], tri[:])
            wo_sb = const.tile([P, 2, D], BF16)
            nc.sync.dma_start(wo_sb[:], wo.rearrange("(c p) n -> p c n", p=P))

            # ---- persistent activations ----
            # qt[:, w, :]: heads (2w, 2w+1) stacked on partitions
            qt = persist.tile([P, 2, S], BF16, tag="qt")
            # K^T zero-padded to full 128-row contraction: [KT;0] and [0;KT]
            kt_lo = persist.tile([P, S], BF16, tag="ktlo")
            kt_hi = persist.tile([P, S], BF16, tag="kthi")
            nc.gpsimd.memset(kt_lo[HD:P, :], 0.0)
            nc.gpsimd.memset(kt_hi[0:HD, :], 0.0)
            # v_aug[:, jc, :]: [V_block (64) | ones (64)]
            v_aug = persist.tile([P, JC, P], BF16, tag="vaug")
            nc.gpsimd.memset(v_aug[:, :, HD:P], 1.0)

            def proj(nt):
                """projections + RoPE for columns [512nt, 512nt+512)"""
                sl = slice(nt * 512, (nt + 1) * 512)
                kv_ps = ps.tile([P, 512], F32, tag="bg", bufs=2, name="kv_ps")
                for k in range(KC):
                    nc.tensor.matmul(
                        kv_ps[:],
                        wkv_sb[:, k, :],
                        xT_sb[k][:, sl],
                        start=(k == 0),
                        stop=(k == KC - 1),
                    )
                kv_raw = sb_tmp.tile([P, 512], BF16, tag="kvraw", name="kv_raw")
                nc.scalar.activation(kv_raw[:], kv_ps[:], AF.Copy)

                # V natural: transpose bf16 V^T blocks straight into v_aug
                for j in range(4):
                    nc.sync.dma_start_transpose(
                        v_aug[:, nt * 4 + j, 0:HD],
                        kv_raw[HD:P, j * P : (j + 1) * P],
                    )

                # K RoPE -> kt_lo rows 0:64, duplicated into kt_hi rows 64:128
                rotk_ps = ps.tile([P, 512], F32, tag="bg", bufs=2, name="rotk_ps")
                nc.tensor.matmul(
                    rotk_ps[0:HD, :],
                    r2t_sb[0:HD, 0:HD],
                    kv_raw[0:HD, :],
                    start=True,
                    stop=True,
                )
                rotk_b = sb_tmp.tile([HD, 512], BF16, tag="rotb", name="rotk_b")
                nc.scalar.activation(rotk_b[:], rotk_ps[0:HD, :], AF.Copy)
                t1k = sb_tmp.tile([HD, 512], BF16, tag="t1", name="t1k")
                nc.vector.tensor_mul(t1k[:], kv_raw[0:HD, :], cos_sb[0:HD, sl])
                t2k = sb_tmp.tile([HD, 512], BF16, tag="t2", name="t2k")
                nc.vector.tensor_mul(t2k[:], rotk_b[:], sin_sb[0:HD, sl])
                nc.vector.tensor_add(kt_lo[0:HD, sl], t1k[:], t2k[:])
                nc.vector.tensor_copy(kt_hi[HD:P, sl], kt_lo[0:HD, sl])

                for w in range(2):
                    q_ps = ps.tile([P, 512], F32, tag="bg", bufs=2, name="q_ps")
                    for k in range(KC):
                        nc.tensor.matmul(
                            q_ps[:],
                            wq_sb[:, k, w * P : (w + 1) * P],
                            xT_sb[k][:, sl],
                            start=(k == 0),
                            stop=(k == KC - 1),
                        )
                    q_raw = sb_tmp.tile([P, 512], BF16, tag="qraw", name="q_raw")
                    nc.scalar.activation(q_raw[:], q_ps[:], AF.Copy)
                    rot_ps = ps.tile([P, 512], F32, tag="bg", bufs=2, name="rot_ps")
                    nc.tensor.matmul(
                        rot_ps[:], r2t_sb[:], q_raw[:], start=True, stop=True
                    )
                    rot_b = sb_tmp.tile([P, 512], BF16, tag="rotb", name="rot_b")
                    nc.scalar.activation(rot_b[:], rot_ps[:], AF.Copy)
                    t1 = sb_tmp.tile([P, 512], BF16, tag="t1", name="t1q")
                    nc.vector.tensor_mul(t1[:], q_raw[:], cos_sb[:, sl])
                    t2 = sb_tmp.tile([P, 512], BF16, tag="t2", name="t2q")
                    nc.vector.tensor_mul(t2[:], rot_b[:], sin_sb[:, sl])
                    nc.vector.tensor_add(qt[:, w, sl], t1[:], t2[:])

            proj(0)
            for nt in range(NT):
                if nt + 1 < NT:
                    proj(nt + 1)

                # ============ attention for i-tile ti = nt ============
                ti = nt
                i0 = ti * 512
                pieces = [(jc, 0) for jc in range(4 * ti)] + [
                    (4 * ti + r, P * r) for r in range(4)
                ]
                ot = []
                for h, kt in ((0, kt_lo), (1, kt_hi)):
                    # av: heads (h, 2+h) in w slices; rows 64:128 = denom
                    av = ps.tile([P, 2, 512], F32, tag="av", bufs=1, name="av")
                    for jc, ls in pieces:
                        n = 512 - ls
                        isl = slice(i0 + ls, i0 + 512)
                        jsl = slice(jc * P, (jc + 1) * P)
                        sc = ps.tile([P, 2, 512], F32, tag="sc", bufs=2, name="sc")
                        for w in range(2):
                            nc.tensor.matmul(
                                sc[:, w, 0:n],
                                kt[:, jsl],
                                qt[:, w, isl],
                                start=True,
                                stop=True,
                            )
                        pt = sb_pt.tile([P, 2, 512], BF16, tag="pt", name="pt")
                        nc.scalar.activation(
                            pt[:, :, 0:n], sc[:, :, 0:n], AF.Exp, scale=0.125
                        )
                        if jc >= 4 * ti:  # diagonal chunk: mask first block
                            nc.vector.tensor_mul(
                                pt[:, :, 0:P], pt[:, :, 0:P], tri2_sb[:]
                            )
                        for w in range(2):
                            nc.tensor.matmul(
                                av[:, w, ls:512],
                                v_aug[:, jc, :],
                                pt[:, w, 0:n],
                                start=(jc == 0),
                                stop=(jc == pieces[-1][0]),
                            )

                    # normalize -> ot chunk h (rows: [head h | head 2+h])
                    rec = sb_tmp.tile([P, 2, 512], F32, tag="rec", name="rec")
                    nc.vector.reciprocal_approx_fast(rec[:], av[:])
                    ot_h = sb_ot.tile([P, 512], BF16, tag=f"ot{h}", name="ot_h")
                    for w in range(2):
                        nc.vector.tensor_mul(
                            ot_h[w * HD : (w + 1) * HD, :],
                            av[0:HD, w, :],
                            rec[HD:P, w, :],
                        )
                    ot.append(ot_h)

                # ============ output projection for i-tile ============
                for ic in range(4):
                    csl = slice(ic * P, (ic + 1) * P)
                    r0 = i0 + ic * P
                    out_sb = sb_out.tile([P, D], BF16, tag="osb", name="out_sb")
                    for ntile in range(2):
                        wo_ps = ps.tile([P, 512], F32, tag="bg", bufs=2, name="wo_ps")
                        nsl = slice(ntile * 512, (ntile + 1) * 512)
                        nc.tensor.matmul(
                            wo_ps[:], ot[0][:, csl], wo_sb[:, 0, nsl],
                            start=True, stop=False,
                        )
                        nc.tensor.matmul(
                            wo_ps[:], ot[1][:, csl], wo_sb[:, 1, nsl],
                            start=False, stop=True,
                        )
                        nc.vector.tensor_copy(out_sb[:, nsl], wo_ps[:])
                    nc.sync.dma_start(out[r0 : r0 + P, :], out_sb[:])

    nc.compile()
    return nc


def _prep_inputs(x, cos, sin, Wq, Wk, Wv, Wo):
    """Build per-core input maps (host-side sharding + layout), all bf16."""
    import ml_dtypes

    bf16 = ml_dtypes.bfloat16

    cosT = np.ascontiguousarray(cos.T)  # (64, 2048)
    sinT = np.ascontiguousarray(sin.T)
    cos2 = np.concatenate([cosT, cosT], axis=0).astype(bf16)
    sin2 = np.concatenate([sinT, sinT], axis=0).astype(bf16)

    # lhsT of rotate_half: rot(q) = R q, r2t = R^T (block-diag over 2 heads)
    r2t = np.zeros((P, P), dtype=np.float32)
    for o in (0, HD):
        for e in range(32):
            r2t[o + e, o + e + 32] = 1.0
        for e in range(32, HD):
            r2t[o + e, o + e - 32] = -1.0

    tri = (np.arange(P)[:, None] <= np.arange(P)[None, :]).astype(bf16)

    xT = [np.ascontiguousarray(x[b].T).astype(bf16) for b in range(B)]

    in_maps = []
    for c in range(N_CORES):
        b, g = c // KV, c % KV
        # wo rows reordered to match ot chunk layout: c0=[h0,h2], c1=[h1,h3]
        wo_g = Wo[g * NH * HD : (g + 1) * NH * HD, :].reshape(NH, HD, D)
        wo_perm = np.concatenate([wo_g[0], wo_g[2], wo_g[1], wo_g[3]], axis=0)
        in_maps.append(
            {
                "xT": xT[b],
                "wq": np.ascontiguousarray(
                    Wq[:, g * NH * HD : (g + 1) * NH * HD]
                ).astype(bf16),
                "wkv": np.ascontiguousarray(
                    np.concatenate(
                        [Wk[:, g * HD : (g + 1) * HD], Wv[:, g * HD : (g + 1) * HD]],
                        axis=1,
                    )
                ).astype(bf16),
                "wo": np.ascontiguousarray(wo_perm).astype(bf16),
                "cos2": cos2,
                "sin2": sin2,
                "r2t": r2t.astype(bf16),
                "tri": tri,
            }
        )
    return in_maps


def kernel(x, cos, sin, Wq, Wk, Wv, Wo):
    x = np.asarray(x, dtype=np.float32)
    cos = np.asarray(cos, dtype=np.float32)
    sin = np.asarray(sin, dtype=np.float32)
    Wq = np.asarray(Wq, dtype=np.float32)
    Wk = np.asarray(Wk, dtype=np.float32)
    Wv = np.asarray(Wv, dtype=np.float32)
    Wo = np.asarray(Wo, dtype=np.float32)

    trace = os.environ.get("TRN_TRACE", "") == "1"
    if trace:
        _install_trace_hook()

    if "nc" not in _cached:
        _cached["nc"] = build_bass()
    nc = _cached["nc"]

    in_maps = _prep_inputs(x, cos, sin, Wq, Wk, Wv, Wo)
    res = run_bass_kernel_spmd(nc, in_maps, list(range(N_CORES)), trace=trace)
    if trace and res.exec_time_ns is not None:
        print(f"HW exec time: {res.exec_time_ns} ns")
        _cached["exec_time_ns"] = res.exec_time_ns
        _cached["trace_path"] = (
            res.instructions_and_trace[1] if res.instructions_and_trace else None
        )

    out = np.zeros((B, S, D), dtype=np.float32)
    for c in range(N_CORES):
        out[c // KV] += np.asarray(res.results[c]["out"], dtype=np.float32)
    return out
